# revision 2
# baseline (speedup 1.0000x reference)
"""Bezier stroke renderer on 8 Trainium2 NeuronCores — canvas-major
depth-packed Bass/Tile SPMD kernel (v2).

Reference: 32 strokes x 16 segments rasterized on a 1024x1024 canvas;
per pixel/segment darkness = clip((2t - dist)/(2t), 0, 1), max over
segments, grid = max(grid, darkness*color) over strokes.

Design:
  - Canvas rows split into 128 bands of 8 rows; each core owns 16
    consecutive bands (one 128-row slab), one band per 8-partition group.
  - Per band, each segment yields a column window [c0,c1] (pad=2t+1).
    For a canvas column c, m(c) = max over the core's bands of the
    number of windows covering c.  Columns are sorted by m desc and the
    packed axis stores, level-major, one slot per (level l, column with
    m>l).  Every group contributes its own window coefficients at each
    slot; empty slots are dead (zero coeffs, zero color).
  - Distance math in the segment tangent frame scaled by 1/(2t), with
    per-band row centering (delta = x - band_x0 in [0,8)) so 2-way fp16
    coefficient splits reach ~2^-22 relative accuracy:
      a  = a1*delta + a2(c)        (tangent coordinate)
      al = a - L/(2t)
      b2q = c2*delta^2 + c1*delta + c0     (= b^2, quadratic in delta)
    TensorE computes a, al, b2q; overshoot mp = max(relu(al), relu(-a))
    (ACT relus, DVE max), mp^2 (DVE), accumulated onto the b2q PSUM bank
    by an identity matmul; ACT takes sqrt; DVE scalar_tensor_tensor
    writes (dd-1)*col_ch into three fp16 channel planes (4x DVE mode).
  - Composite: level-major layout makes each level a prefix of the
    sorted columns, so min-compositing is mmax-1 in-place fp16 MIN ops
    per channel.  Output = level-0 block, fp16; host does relu(-x),
    scatter to canvas, fp32.
"""

import sys
import types
import contextlib
import ctypes

sys.path.insert(0, "/opt/trn_rl_repo")

import numpy as np

G = 1024
P = 16
N = 32
N_CORES = 8
BH = 16                # band height (rows)
NB = G // BH           # 64 bands
NG = 128 // BH         # 8 bands (groups) per core
SUPER = 512            # columns per PSUM chunk (1 bank per quantity)
COLBUF_BCAST = True    # expand colors on-chip via 0-stride DMA (else HBM full)

_PROG_CACHE = {}
_HOOK_INSTALLED = False


def _install_ntff_hook():
    global _HOOK_INSTALLED
    if _HOOK_INSTALLED:
        return
    _HOOK_INSTALLED = True
    try:
        import antenv
        mod = types.ModuleType("antenv.axon_hooks")
        holder = [None]
        mod.set_axon_ntff_profile_hook = lambda h: holder.__setitem__(0, h)
        mod.get_axon_ntff_profile_hook = lambda: holder[0]
        sys.modules["antenv.axon_hooks"] = mod
        antenv.axon_hooks = mod

        lib = ctypes.CDLL("/opt/axon/libaxon_pjrt.so")
        if not hasattr(lib, "axon_start_nrt_profile"):
            return
        lib.axon_start_nrt_profile.argtypes = [
            ctypes.POINTER(ctypes.c_int64),
            ctypes.c_size_t,
        ]
        lib.axon_start_nrt_profile.restype = ctypes.c_int64
        lib.axon_stop_nrt_profile.argtypes = [ctypes.c_char_p]
        lib.axon_stop_nrt_profile.restype = ctypes.c_int64

        @contextlib.contextmanager
        def _hook(output_dir, device_ids):
            import jax
            jax.devices()
            if device_ids:
                ids = (ctypes.c_int64 * len(device_ids))(*device_ids)
                rc = lib.axon_start_nrt_profile(ids, len(device_ids))
            else:
                rc = lib.axon_start_nrt_profile(None, 0)
            if rc != 0:
                raise RuntimeError(f"axon_start_nrt_profile rc={rc}")
            try:
                yield
            finally:
                n = lib.axon_stop_nrt_profile(str(output_dir).encode())
                print(f"profile: {n} file(s) written to {output_dir}",
                      file=sys.stderr)

        mod.set_axon_ntff_profile_hook(_hook)
    except Exception:
        pass


# ---------------------------------------------------------------- host side

def _bezier_weights_f32(p):
    t = np.arange(p, dtype=np.float64)
    w1 = (p - t) ** 3 / p ** 3
    w2 = 3 * (p - t) ** 2 * t / p ** 3
    w3 = 3 * (p - t) * t ** 2 / p ** 3
    w4 = t ** 3 / p ** 3
    return np.stack([w1, w2, w3, w4]).astype(np.float32)


def _polylines(strokes):
    W = _bezier_weights_f32(P)
    s = strokes.astype(np.float32)
    pts, derivs = s[:, :, :2], s[:, :, 2:]
    p1, p2 = pts[:, :-1], (pts + derivs)[:, :-1]
    p3, p4 = (pts - derivs)[:, 1:], pts[:, 1:]
    cp = np.stack([p1, p2, p3, p4], axis=3)
    sp = np.einsum("nsdk,kp->nspd", cp, W).astype(np.float32)
    sp = sp.reshape(s.shape[0], -1, 2)
    poly = np.concatenate([sp, pts[:, -1:, :]], axis=1).astype(np.float32)
    return (poly * np.float32(G)).astype(np.float64)


def _build_layout(strokes, thicknesses, colors):
    """Window extraction, per-core depth profiles, harmonized level
    layout, and all device tables."""
    poly = _polylines(strokes)
    t = np.maximum(thicknesses.astype(np.float32) * np.float32(2.0)
                   + np.float32(0.5), np.float32(0.5))[:, 0]
    col = np.clip(colors.astype(np.float32), 0.0, 1.0)
    t64 = t.astype(np.float64)
    pad = 2.0 * t64 + 1.0

    # windows per band: list of (n, iseg, c0, c1)
    wins = [[] for _ in range(NB)]
    for n in range(N):
        pn = poly[n]
        for i in range(P):
            v, w = pn[i], pn[i + 1]
            xlo, xhi = min(v[0], w[0]) - pad[n], max(v[0], w[0]) + pad[n]
            b0 = max(0, int(np.floor(xlo / BH)))
            b1 = min(NB - 1, int(np.floor(xhi / BH)))
            dx = w[0] - v[0]
            for b in range(b0, b1 + 1):
                x0, x1 = BH * b, BH * b + BH - 1
                lo_x, hi_x = x0 - pad[n], x1 + pad[n]
                if abs(dx) < 1e-12:
                    if v[0] < lo_x or v[0] > hi_x:
                        continue
                    s0, s1 = 0.0, 1.0
                else:
                    sa, sb = (lo_x - v[0]) / dx, (hi_x - v[0]) / dx
                    s0 = max(0.0, min(sa, sb))
                    s1 = min(1.0, max(sa, sb))
                    if s0 > s1:
                        continue
                ya = v[1] + s0 * (w[1] - v[1])
                yb = v[1] + s1 * (w[1] - v[1])
                c0 = max(0.0, min(ya, yb) - pad[n])
                c1 = min(G - 1.0, max(ya, yb) + pad[n])
                if c1 < c0:
                    continue
                wins[b].append((n, i, int(np.floor(c0)), int(np.ceil(c1))))

    # per-band, per-column window lists
    colwins = [[[] for _ in range(G)] for _ in range(NB)]
    for b in range(NB):
        for (n, i, c0, c1) in wins[b]:
            for c in range(c0, c1 + 1):
                colwins[b][c].append((n, i))

    # band depth profiles; assign bands to cores to minimize the
    # harmonized packed width (hill-climb from consecutive slabs)
    band_d = np.zeros((NB, G), np.int32)
    for b in range(NB):
        for c in range(G):
            band_d[b, c] = len(colwins[b][c])

    assign = np.arange(NB).reshape(N_CORES, NG)

    def harmonized_wp(asg):
        prof = np.sort(band_d[asg].max(axis=1), axis=1)[:, ::-1]
        return int(prof.max(axis=0).sum())

    rng = np.random.default_rng(12345)
    best = harmonized_wp(assign)
    best_assign = assign.copy()
    cur = best
    temp = 60.0
    for it in range(40000):
        temp *= 0.99985
        c1i, c2i = rng.integers(0, N_CORES, 2)
        if c1i == c2i:
            continue
        g1, g2 = rng.integers(0, NG, 2)
        assign[c1i, g1], assign[c2i, g2] = assign[c2i, g2], assign[c1i, g1]
        cand = harmonized_wp(assign)
        if cand <= cur or rng.random() < np.exp(-(cand - cur) / max(temp, 1e-9)):
            cur = cand
            if cand < best:
                best = cand
                best_assign = assign.copy()
        else:
            assign[c1i, g1], assign[c2i, g2] = (assign[c2i, g2],
                                                assign[c1i, g1])
    assign = best_assign

    m_core = np.zeros((N_CORES, G), np.int32)
    for cidx in range(N_CORES):
        m_core[cidx] = band_d[assign[cidx]].max(axis=0)

    # per-core column order: sort by m desc (stable)
    order = [np.argsort(-m_core[c], kind="stable") for c in range(N_CORES)]
    sorted_m = np.stack([m_core[c][order[c]] for c in range(N_CORES)])
    common = sorted_m.max(axis=0)           # harmonized profile
    ncols = int((common > 0).sum())
    mmax = int(common.max())
    # level lengths: len_l = #cols with common > l
    lens = [int((common > l).sum()) for l in range(mmax)]
    offs = np.concatenate([[0], np.cumsum(lens)]).astype(np.int64)
    Wp = int(offs[-1])
    Wp_pad = ((Wp + SUPER - 1) // SUPER) * SUPER

    # device tables per core
    KA = 4 * NG            # rows for a and al quantities
    KB = 6 * NG            # rows for b^2 quantity
    in_maps = []
    for cidx in range(N_CORES):
        ordc = order[cidx]
        # entry collection: (g, level, sortidx, n, iseg, canvascol)
        gs, ls, sis, ns, isegs, cs = [], [], [], [], [], []
        for g in range(NG):
            b = int(assign[cidx][g])
            for si in range(ncols):
                c = int(ordc[si])
                lst = colwins[b][c]
                for l, (n, i) in enumerate(lst):
                    gs.append(g); ls.append(l); sis.append(si)
                    ns.append(n); isegs.append(i); cs.append(c)
        gs = np.array(gs); ls = np.array(ls); sis = np.array(sis)
        ns = np.array(ns); isegs = np.array(isegs); cs = np.array(cs, np.float64)
        pos = offs[ls] + sis               # packed position per entry

        v = poly[ns, isegs]                # (E,2)
        w = poly[ns, isegs + 1]
        dxy = w - v
        L = np.hypot(dxy[:, 0], dxy[:, 1])
        safe = L > 1e-9
        taux = np.where(safe, dxy[:, 0] / np.where(safe, L, 1.0), 1.0)
        tauy = np.where(safe, dxy[:, 1] / np.where(safe, L, 1.0), 0.0)
        Leff = np.where(safe, L, 0.0)
        nux, nuy = -tauy, taux
        i2t = 1.0 / (2.0 * t64[ns])
        x0 = (assign[cidx][gs] * BH).astype(np.float64)  # band start row

        av = v[:, 0] * taux + v[:, 1] * tauy
        bv = v[:, 0] * nux + v[:, 1] * nuy
        a1 = taux * i2t
        a2 = (cs * tauy - av) * i2t + a1 * x0            # centered const
        al2 = a2 - Leff * i2t
        b1 = nux * i2t
        b20 = (cs * nuy - bv) * i2t + b1 * x0
        c2 = b1 * b1
        c1 = 2.0 * b1 * b20
        c0_ = b20 * b20

        def split2(vv):
            h = vv.astype(np.float16)
            lo = (vv - h.astype(np.float64)).astype(np.float16)
            return h, lo

        # rtall rows: [0:KA] a-L coeffs, [KA:2KA] NEGATED a coeffs (so a
        # single +relu serves both), [2KA:2KA+KB] b^2 quadratic coeffs
        rtall = np.zeros((2 * KA + KB, Wp_pad), np.float16)
        colbc = np.zeros((3, NG, Wp_pad), np.float16)

        a1h, a1l = split2(a1)
        na1h, na1l = split2(-a1)
        na2h, na2l = split2(-a2)
        al2h, al2l = split2(al2)
        c2h, c2l = split2(c2)
        c1h, c1l = split2(c1)
        c0h, c0l = split2(c0_)
        rtall[4 * gs + 0, pos] = a1h
        rtall[4 * gs + 1, pos] = a1l
        rtall[4 * gs + 2, pos] = al2h
        rtall[4 * gs + 3, pos] = al2l
        rtall[KA + 4 * gs + 0, pos] = na1h
        rtall[KA + 4 * gs + 1, pos] = na1l
        rtall[KA + 4 * gs + 2, pos] = na2h
        rtall[KA + 4 * gs + 3, pos] = na2l
        rtall[2 * KA + 6 * gs + 0, pos] = c2h
        rtall[2 * KA + 6 * gs + 1, pos] = c2l
        rtall[2 * KA + 6 * gs + 2, pos] = c1h
        rtall[2 * KA + 6 * gs + 3, pos] = c1l
        rtall[2 * KA + 6 * gs + 4, pos] = c0h
        rtall[2 * KA + 6 * gs + 5, pos] = c0l
        colv = col[ns]                      # (E,3) f32
        for ch in range(3):
            colbc[ch, gs, pos] = colv[:, ch].astype(np.float16)

        # lhsT tables
        dl = np.arange(128) % BH            # delta per partition
        xt_a = np.zeros((KA, 128), np.float16)
        xt_b2 = np.zeros((KB, 128), np.float16)
        for g in range(NG):
            sel = np.zeros(128, np.float16)
            sel[g * BH:(g + 1) * BH] = 1.0
            dsel = (dl * sel).astype(np.float16)
            xt_a[4 * g + 0] = dsel
            xt_a[4 * g + 1] = dsel
            xt_a[4 * g + 2] = sel
            xt_a[4 * g + 3] = sel
            xt_b2[6 * g + 0] = (dl * dl * sel).astype(np.float16)
            xt_b2[6 * g + 1] = xt_b2[6 * g + 0]
            xt_b2[6 * g + 2] = dsel
            xt_b2[6 * g + 3] = dsel
            xt_b2[6 * g + 4] = sel
            xt_b2[6 * g + 5] = sel

        im = {"rtall": rtall}
        xts = np.zeros((128, 384), np.float16)
        xts[:, 0:128] = np.eye(128, dtype=np.float16)
        xts[0:KA, 128:256] = xt_a
        xts[KA:2 * KA, 128:256] = xt_a
        xts[2 * KA:2 * KA + KB, 256:384] = xt_b2
        im["xts"] = xts
        # colb: [128, 3*Wp] channel-major full expansion
        im["colb"] = np.repeat(colbc, BH, axis=1).transpose(1, 0, 2).reshape(
            128, 3 * Wp_pad)
        in_maps.append(im)

    # adaptive epsilon: the expanded b^2 quadratic can round slightly
    # negative (sqrt would NaN).  Emulate the matmul, find the most
    # negative excursion, and fold a safely-larger offset into the c0
    # rows.  Costs sqrt(eps) extra darkness only where dist ~ 0.
    b2qmin = 0.0
    for im in in_maps:
        xt = im["xts"][2 * KA:2 * KA + KB, 256:384].astype(np.float32)
        rb = im["rtall"][2 * KA:].astype(np.float32)
        b2qmin = min(b2qmin, float((xt.T @ rb).min()))
    eps = max(2e-5, -1.5 * b2qmin)
    epsh = np.float16(eps)
    epsl = np.float16(eps - np.float64(epsh))
    for im in in_maps:
        rt = im["rtall"]
        # add eps (split exactly) onto every live c0 pair; dead slots keep
        # zero color so a nonzero dd there is harmless, but restrict to
        # live ones anyway via the existing h-part nonzero mask union
        for g in range(NG):
            r = 2 * KA + 6 * g
            h64 = rt[r + 4].astype(np.float64) + float(epsh)
            l64 = rt[r + 5].astype(np.float64) + float(epsl)
            rt[r + 4] = h64.astype(np.float16)
            rt[r + 5] = (h64 - rt[r + 4].astype(np.float64)
                         + l64).astype(np.float16)

    meta = {
        "Wp_pad": Wp_pad, "ncols": ncols, "mmax": mmax,
        "lens": lens, "offs": offs.tolist(),
        "order": order, "m_core": m_core, "assign": assign,
    }
    return in_maps, meta


# ---------------------------------------------------------------- bass side

def _build_program(Wp_pad, ncols, mmax, lens, offs):
    import concourse.bacc as bacc
    import concourse.mybir as mybir
    from concourse import tile

    f16 = mybir.dt.float16
    f32 = mybir.dt.float32
    AF = mybir.ActivationFunctionType
    OP = mybir.AluOpType
    KA = 4 * NG
    KB = 6 * NG
    NSC = Wp_pad // SUPER

    KR = 2 * KA + KB
    nc = bacc.Bacc("TRN2", target_bir_lowering=False, debug=False,
                   num_devices=N_CORES)
    xts_d = nc.dram_tensor("xts", [128, 384], f16, kind="ExternalInput").ap()
    rtall_d = nc.dram_tensor("rtall", [KR, Wp_pad], f16,
                             kind="ExternalInput").ap()
    colb_d = nc.dram_tensor("colb", [128, 3 * Wp_pad], f16,
                            kind="ExternalInput").ap()
    out_d = nc.dram_tensor("out", [128, 3 * ncols], f16,
                           kind="ExternalOutput").ap()

    with tile.TileContext(nc) as tc:
        with (
            tc.tile_pool(name="const", bufs=1) as constp,
            tc.tile_pool(name="work", bufs=6) as workp,
            tc.tile_pool(name="psA", bufs=2, space="PSUM") as psumA,
            tc.tile_pool(name="psB", bufs=4, space="PSUM") as psumB,
        ):
            xts = constp.tile([128, 384], f16)
            ident = xts[:, 0:128]
            xt_al = xts[0:KA, 128:256]
            xt_an = xts[KA:2 * KA, 128:256]
            xt_b2 = xts[2 * KA:2 * KA + KB, 256:384]

            rtall = constp.tile([KR, Wp_pad], f16)
            colb = constp.tile([128, 3 * Wp_pad], f16)
            vint = constp.tile([128, 3 * Wp_pad], f16)

            # preload the combined relu/sqrt ACT table before data arrives
            dmy = workp.tile([1, 16], f16, tag="dmy")
            nc.gpsimd.memset(dmy[:], 0.0)
            nc.scalar.activation(dmy[:], dmy[:], AF.Sqrt)

            # warm the PE clock gate on a junk tile during the DMA dead
            # zone so real matmuls start at full clock
            junk = constp.tile([128, 512], f16)
            nc.gpsimd.memset(junk[:], 0.0)
            pwarm = psumA.tile([128, 1024], f32, tag="pp")
            for _ in range(7):
                nc.tensor.matmul(pwarm[:, 0:512], junk[:, 0:128], junk[:])

            # single SP issue stream in consumption order: chunk-0 rt
            # slice, lhsT, chunk-0 colors, then the rest per chunk
            colb3 = colb[:].rearrange("p (c w) -> p c w", c=3)
            colb3_d = colb_d[:].rearrange("p (c w) -> p c w", c=3)
            nc.sync.dma_start(rtall[:, 0:SUPER], rtall_d[:, 0:SUPER])
            nc.sync.dma_start(xts[:], xts_d[:])
            nc.sync.dma_start(colb3[:, :, 0:SUPER], colb3_d[:, :, 0:SUPER])
            for sc in range(1, NSC):
                sl = slice(sc * SUPER, (sc + 1) * SUPER)
                nc.sync.dma_start(rtall[:, sl], rtall_d[:, sl])
                nc.sync.dma_start(colb3[:, :, sl], colb3_d[:, :, sl])

            # fold l becomes runnable once the chunk covering its src
            # range completes; emit right after that chunk's mults unless
            # it is the last chunk (keep the final critical tail clean)
            fold_after = {}
            for l in range(1, mmax):
                hi = offs[l] + lens[l]
                rdy = (hi - 1) // SUPER
                if rdy <= NSC - 2:
                    fold_after.setdefault(rdy, []).append(l)

            for sc in range(NSC):
                sl = slice(sc * SUPER, (sc + 1) * SUPER)
                palpa = psumA.tile([128, 1024], f32, tag="pp")
                pb2 = psumB.tile([128, SUPER], f32, tag="pb")
                nc.tensor.matmul(palpa[:, 0:512], xt_al, rtall[0:KA, sl])
                nc.tensor.matmul(palpa[:, 512:1024], xt_an,
                                 rtall[KA:2 * KA, sl])
                nc.tensor.matmul(pb2[:], xt_b2, rtall[2 * KA:KR, sl],
                                 start=True, stop=False)

                q12 = workp.tile([128, 1024], f16, tag="q12")
                mp = workp.tile([128, SUPER], f16, tag="mp")
                mp2 = workp.tile([128, SUPER], f16, tag="mp2")
                dd = workp.tile([128, SUPER], f16, tag="dd")

                nc.scalar.activation(q12[:], palpa[:], AF.Relu)
                nc.vector.tensor_tensor(mp[:], q12[:, 0:512],
                                        q12[:, 512:1024], op=OP.max)
                nc.vector.tensor_tensor(mp2[:], mp[:], mp[:], op=OP.mult)
                nc.tensor.matmul(pb2[:], ident, mp2[:],
                                 start=False, stop=True)
                nc.scalar.activation(dd[:], pb2[:], AF.Sqrt)
                ddm1 = workp.tile([128, SUPER], f16, tag="ddm1")
                nc.vector.tensor_scalar(ddm1[:], dd[:], 1.0, None,
                                        op0=OP.subtract)
                for ch in range(3):
                    nc.vector.tensor_tensor(
                        vint[:, ch * Wp_pad + sc * SUPER:
                             ch * Wp_pad + (sc + 1) * SUPER],
                        ddm1[:],
                        colb[:, ch * Wp_pad + sc * SUPER:
                             ch * Wp_pad + (sc + 1) * SUPER],
                        op=OP.mult)
                # ready folds slot in behind this chunk's mults
                for l in fold_after.get(sc, []):
                    w = lens[l]
                    o = offs[l]
                    for ch in range(3):
                        base = ch * Wp_pad
                        nc.vector.tensor_tensor(
                            vint[:, base:base + w], vint[:, base:base + w],
                            vint[:, base + o:base + o + w], op=OP.min)

            # remaining folds (ranges touching the last chunk); emit each
            # channel's output DMA as soon as its folds are done
            done_l = {l for ls in fold_after.values() for l in ls}
            rest = [l for l in range(1, mmax) if l not in done_l]
            for ch in range(3):
                base = ch * Wp_pad
                for l in rest:
                    w = lens[l]
                    o = offs[l]
                    nc.vector.tensor_tensor(
                        vint[:, base:base + w], vint[:, base:base + w],
                        vint[:, base + o:base + o + w], op=OP.min)
                eng = (nc.sync, nc.scalar, nc.sync)[ch]
                eng.dma_start(
                    out_d[:, ch * ncols:(ch + 1) * ncols],
                    vint[:, base:base + ncols])

    nc.compile()
    return nc


# ---------------------------------------------------------------- entry

def kernel(strokes, thicknesses, colors):
    _install_ntff_hook()
    from concourse.bass_utils import run_bass_kernel_spmd

    strokes = np.asarray(strokes)
    thicknesses = np.asarray(thicknesses)
    colors = np.asarray(colors)

    in_maps, meta = _build_layout(strokes, thicknesses, colors)
    key = (meta["Wp_pad"], meta["ncols"], meta["mmax"],
           tuple(meta["lens"]))
    if key not in _PROG_CACHE:
        _PROG_CACHE[key] = _build_program(
            meta["Wp_pad"], meta["ncols"], meta["mmax"],
            meta["lens"], meta["offs"])
    nc = _PROG_CACHE[key]

    res = run_bass_kernel_spmd(nc, in_maps, list(range(N_CORES)))

    ncols = meta["ncols"]
    out = np.zeros((3, G, G), np.float32)
    rows = np.arange(128)
    for cidx in range(N_CORES):
        mc = meta["m_core"][cidx]
        ordc = meta["order"][cidx][:ncols]
        real = mc[ordc] > 0
        cols_real = ordc[real]
        row_ids = (meta["assign"][cidx][rows // BH] * BH + rows % BH)
        for ch in range(3):
            plane = res.results[cidx]["out"][:, ch * ncols:(ch + 1) * ncols]
            vals = np.maximum(0.0, -plane[:, real].astype(np.float32))
            out[ch][np.ix_(row_ids, cols_real)] = vals
    return out


if __name__ == "__main__":
    rng = np.random.default_rng(0)
    s = rng.random((N, 2, 4), np.float32)
    th = rng.random((N, 1), np.float32)
    co = rng.random((N, 3), np.float32)
    g = kernel(s, th, co)
    print("out", g.shape, g.dtype, g.min(), g.max())


# revision 3
# speedup vs baseline: 1.0027x; 1.0027x over previous
"""Bezier stroke renderer on 8 Trainium2 NeuronCores — canvas-major
depth-packed Bass/Tile SPMD kernel (v2).

Reference: 32 strokes x 16 segments rasterized on a 1024x1024 canvas;
per pixel/segment darkness = clip((2t - dist)/(2t), 0, 1), max over
segments, grid = max(grid, darkness*color) over strokes.

Design:
  - Canvas rows split into 128 bands of 8 rows; each core owns 16
    consecutive bands (one 128-row slab), one band per 8-partition group.
  - Per band, each segment yields a column window [c0,c1] (pad=2t+1).
    For a canvas column c, m(c) = max over the core's bands of the
    number of windows covering c.  Columns are sorted by m desc and the
    packed axis stores, level-major, one slot per (level l, column with
    m>l).  Every group contributes its own window coefficients at each
    slot; empty slots are dead (zero coeffs, zero color).
  - Distance math in the segment tangent frame scaled by 1/(2t), with
    per-band row centering (delta = x - band_x0 in [0,8)) so 2-way fp16
    coefficient splits reach ~2^-22 relative accuracy:
      a  = a1*delta + a2(c)        (tangent coordinate)
      al = a - L/(2t)
      b2q = c2*delta^2 + c1*delta + c0     (= b^2, quadratic in delta)
    TensorE computes a, al, b2q; overshoot mp = max(relu(al), relu(-a))
    (ACT relus, DVE max), mp^2 (DVE), accumulated onto the b2q PSUM bank
    by an identity matmul; ACT takes sqrt; DVE scalar_tensor_tensor
    writes (dd-1)*col_ch into three fp16 channel planes (4x DVE mode).
  - Composite: level-major layout makes each level a prefix of the
    sorted columns, so min-compositing is mmax-1 in-place fp16 MIN ops
    per channel.  Output = level-0 block, fp16; host does relu(-x),
    scatter to canvas, fp32.
"""

import sys
import types
import contextlib
import ctypes

sys.path.insert(0, "/opt/trn_rl_repo")

import numpy as np

G = 1024
P = 16
N = 32
N_CORES = 8
BH = 16                # band height (rows)
NB = G // BH           # 64 bands
NG = 128 // BH         # 8 bands (groups) per core
SUPER = 512            # columns per PSUM chunk (1 bank per quantity)
COLBUF_BCAST = True    # expand colors on-chip via 0-stride DMA (else HBM full)

_PROG_CACHE = {}
_HOOK_INSTALLED = False


def _install_ntff_hook():
    global _HOOK_INSTALLED
    if _HOOK_INSTALLED:
        return
    _HOOK_INSTALLED = True
    try:
        import antenv
        mod = types.ModuleType("antenv.axon_hooks")
        holder = [None]
        mod.set_axon_ntff_profile_hook = lambda h: holder.__setitem__(0, h)
        mod.get_axon_ntff_profile_hook = lambda: holder[0]
        sys.modules["antenv.axon_hooks"] = mod
        antenv.axon_hooks = mod

        lib = ctypes.CDLL("/opt/axon/libaxon_pjrt.so")
        if not hasattr(lib, "axon_start_nrt_profile"):
            return
        lib.axon_start_nrt_profile.argtypes = [
            ctypes.POINTER(ctypes.c_int64),
            ctypes.c_size_t,
        ]
        lib.axon_start_nrt_profile.restype = ctypes.c_int64
        lib.axon_stop_nrt_profile.argtypes = [ctypes.c_char_p]
        lib.axon_stop_nrt_profile.restype = ctypes.c_int64

        @contextlib.contextmanager
        def _hook(output_dir, device_ids):
            import jax
            jax.devices()
            if device_ids:
                ids = (ctypes.c_int64 * len(device_ids))(*device_ids)
                rc = lib.axon_start_nrt_profile(ids, len(device_ids))
            else:
                rc = lib.axon_start_nrt_profile(None, 0)
            if rc != 0:
                raise RuntimeError(f"axon_start_nrt_profile rc={rc}")
            try:
                yield
            finally:
                n = lib.axon_stop_nrt_profile(str(output_dir).encode())
                print(f"profile: {n} file(s) written to {output_dir}",
                      file=sys.stderr)

        mod.set_axon_ntff_profile_hook(_hook)
    except Exception:
        pass


# ---------------------------------------------------------------- host side

def _bezier_weights_f32(p):
    t = np.arange(p, dtype=np.float64)
    w1 = (p - t) ** 3 / p ** 3
    w2 = 3 * (p - t) ** 2 * t / p ** 3
    w3 = 3 * (p - t) * t ** 2 / p ** 3
    w4 = t ** 3 / p ** 3
    return np.stack([w1, w2, w3, w4]).astype(np.float32)


def _polylines(strokes):
    W = _bezier_weights_f32(P)
    s = strokes.astype(np.float32)
    pts, derivs = s[:, :, :2], s[:, :, 2:]
    p1, p2 = pts[:, :-1], (pts + derivs)[:, :-1]
    p3, p4 = (pts - derivs)[:, 1:], pts[:, 1:]
    cp = np.stack([p1, p2, p3, p4], axis=3)
    sp = np.einsum("nsdk,kp->nspd", cp, W).astype(np.float32)
    sp = sp.reshape(s.shape[0], -1, 2)
    poly = np.concatenate([sp, pts[:, -1:, :]], axis=1).astype(np.float32)
    return (poly * np.float32(G)).astype(np.float64)


def _build_layout(strokes, thicknesses, colors):
    """Window extraction, per-core depth profiles, harmonized level
    layout, and all device tables."""
    poly = _polylines(strokes)
    t = np.maximum(thicknesses.astype(np.float32) * np.float32(2.0)
                   + np.float32(0.5), np.float32(0.5))[:, 0]
    col = np.clip(colors.astype(np.float32), 0.0, 1.0)
    t64 = t.astype(np.float64)
    pad = 2.0 * t64 + 1.0

    # windows per band: list of (n, iseg, c0, c1)
    wins = [[] for _ in range(NB)]
    for n in range(N):
        pn = poly[n]
        for i in range(P):
            v, w = pn[i], pn[i + 1]
            xlo, xhi = min(v[0], w[0]) - pad[n], max(v[0], w[0]) + pad[n]
            b0 = max(0, int(np.floor(xlo / BH)))
            b1 = min(NB - 1, int(np.floor(xhi / BH)))
            dx = w[0] - v[0]
            for b in range(b0, b1 + 1):
                x0, x1 = BH * b, BH * b + BH - 1
                lo_x, hi_x = x0 - pad[n], x1 + pad[n]
                if abs(dx) < 1e-12:
                    if v[0] < lo_x or v[0] > hi_x:
                        continue
                    s0, s1 = 0.0, 1.0
                else:
                    sa, sb = (lo_x - v[0]) / dx, (hi_x - v[0]) / dx
                    s0 = max(0.0, min(sa, sb))
                    s1 = min(1.0, max(sa, sb))
                    if s0 > s1:
                        continue
                ya = v[1] + s0 * (w[1] - v[1])
                yb = v[1] + s1 * (w[1] - v[1])
                c0 = max(0.0, min(ya, yb) - pad[n])
                c1 = min(G - 1.0, max(ya, yb) + pad[n])
                if c1 < c0:
                    continue
                wins[b].append((n, i, int(np.floor(c0)), int(np.ceil(c1))))

    # per-band, per-column window lists
    colwins = [[[] for _ in range(G)] for _ in range(NB)]
    for b in range(NB):
        for (n, i, c0, c1) in wins[b]:
            for c in range(c0, c1 + 1):
                colwins[b][c].append((n, i))

    # band depth profiles; assign bands to cores to minimize the
    # harmonized packed width (hill-climb from consecutive slabs)
    band_d = np.zeros((NB, G), np.int32)
    for b in range(NB):
        for c in range(G):
            band_d[b, c] = len(colwins[b][c])

    assign = np.arange(NB).reshape(N_CORES, NG)

    def harmonized_wp(asg):
        prof = np.sort(band_d[asg].max(axis=1), axis=1)[:, ::-1]
        return int(prof.max(axis=0).sum())

    rng = np.random.default_rng(12345)
    best = harmonized_wp(assign)
    best_assign = assign.copy()
    cur = best
    temp = 60.0
    for it in range(40000):
        temp *= 0.99985
        c1i, c2i = rng.integers(0, N_CORES, 2)
        if c1i == c2i:
            continue
        g1, g2 = rng.integers(0, NG, 2)
        assign[c1i, g1], assign[c2i, g2] = assign[c2i, g2], assign[c1i, g1]
        cand = harmonized_wp(assign)
        if cand <= cur or rng.random() < np.exp(-(cand - cur) / max(temp, 1e-9)):
            cur = cand
            if cand < best:
                best = cand
                best_assign = assign.copy()
        else:
            assign[c1i, g1], assign[c2i, g2] = (assign[c2i, g2],
                                                assign[c1i, g1])
    assign = best_assign

    m_core = np.zeros((N_CORES, G), np.int32)
    for cidx in range(N_CORES):
        m_core[cidx] = band_d[assign[cidx]].max(axis=0)

    # per-core column order: sort by m desc (stable)
    order = [np.argsort(-m_core[c], kind="stable") for c in range(N_CORES)]
    sorted_m = np.stack([m_core[c][order[c]] for c in range(N_CORES)])
    common = sorted_m.max(axis=0)           # harmonized profile
    ncols = int((common > 0).sum())
    mmax = int(common.max())
    # level lengths: len_l = #cols with common > l
    lens = [int((common > l).sum()) for l in range(mmax)]
    offs = np.concatenate([[0], np.cumsum(lens)]).astype(np.int64)
    Wp = int(offs[-1])
    Wp_pad = ((Wp + SUPER - 1) // SUPER) * SUPER

    # device tables per core
    KA = 4 * NG            # rows for a and al quantities
    KB = 6 * NG            # rows for b^2 quantity
    in_maps = []
    for cidx in range(N_CORES):
        ordc = order[cidx]
        # entry collection: (g, level, sortidx, n, iseg, canvascol)
        gs, ls, sis, ns, isegs, cs = [], [], [], [], [], []
        for g in range(NG):
            b = int(assign[cidx][g])
            for si in range(ncols):
                c = int(ordc[si])
                lst = colwins[b][c]
                for l, (n, i) in enumerate(lst):
                    gs.append(g); ls.append(l); sis.append(si)
                    ns.append(n); isegs.append(i); cs.append(c)
        gs = np.array(gs); ls = np.array(ls); sis = np.array(sis)
        ns = np.array(ns); isegs = np.array(isegs); cs = np.array(cs, np.float64)
        pos = offs[ls] + sis               # packed position per entry

        v = poly[ns, isegs]                # (E,2)
        w = poly[ns, isegs + 1]
        dxy = w - v
        L = np.hypot(dxy[:, 0], dxy[:, 1])
        safe = L > 1e-9
        taux = np.where(safe, dxy[:, 0] / np.where(safe, L, 1.0), 1.0)
        tauy = np.where(safe, dxy[:, 1] / np.where(safe, L, 1.0), 0.0)
        Leff = np.where(safe, L, 0.0)
        nux, nuy = -tauy, taux
        i2t = 1.0 / (2.0 * t64[ns])
        x0 = (assign[cidx][gs] * BH).astype(np.float64)  # band start row

        av = v[:, 0] * taux + v[:, 1] * tauy
        bv = v[:, 0] * nux + v[:, 1] * nuy
        a1 = taux * i2t
        a2 = (cs * tauy - av) * i2t + a1 * x0            # centered const
        al2 = a2 - Leff * i2t
        b1 = nux * i2t
        b20 = (cs * nuy - bv) * i2t + b1 * x0
        c2 = b1 * b1
        c1 = 2.0 * b1 * b20
        c0_ = b20 * b20

        def split2(vv):
            h = vv.astype(np.float16)
            lo = (vv - h.astype(np.float64)).astype(np.float16)
            return h, lo

        # rtall rows: [0:KA] a-L coeffs, [KA:2KA] NEGATED a coeffs (so a
        # single +relu serves both), [2KA:2KA+KB] b^2 quadratic coeffs
        rtall = np.zeros((2 * KA + KB, Wp_pad), np.float16)
        colbc = np.zeros((3, NG, Wp_pad), np.float16)

        a1h, a1l = split2(a1)
        na1h, na1l = split2(-a1)
        na2h, na2l = split2(-a2)
        al2h, al2l = split2(al2)
        c2h, c2l = split2(c2)
        c1h, c1l = split2(c1)
        c0h, c0l = split2(c0_)
        rtall[4 * gs + 0, pos] = a1h
        rtall[4 * gs + 1, pos] = a1l
        rtall[4 * gs + 2, pos] = al2h
        rtall[4 * gs + 3, pos] = al2l
        rtall[KA + 4 * gs + 0, pos] = na1h
        rtall[KA + 4 * gs + 1, pos] = na1l
        rtall[KA + 4 * gs + 2, pos] = na2h
        rtall[KA + 4 * gs + 3, pos] = na2l
        rtall[2 * KA + 6 * gs + 0, pos] = c2h
        rtall[2 * KA + 6 * gs + 1, pos] = c2l
        rtall[2 * KA + 6 * gs + 2, pos] = c1h
        rtall[2 * KA + 6 * gs + 3, pos] = c1l
        rtall[2 * KA + 6 * gs + 4, pos] = c0h
        rtall[2 * KA + 6 * gs + 5, pos] = c0l
        colv = col[ns]                      # (E,3) f32
        for ch in range(3):
            colbc[ch, gs, pos] = colv[:, ch].astype(np.float16)

        # lhsT tables
        dl = np.arange(128) % BH            # delta per partition
        xt_a = np.zeros((KA, 128), np.float16)
        xt_b2 = np.zeros((KB, 128), np.float16)
        for g in range(NG):
            sel = np.zeros(128, np.float16)
            sel[g * BH:(g + 1) * BH] = 1.0
            dsel = (dl * sel).astype(np.float16)
            xt_a[4 * g + 0] = dsel
            xt_a[4 * g + 1] = dsel
            xt_a[4 * g + 2] = sel
            xt_a[4 * g + 3] = sel
            xt_b2[6 * g + 0] = (dl * dl * sel).astype(np.float16)
            xt_b2[6 * g + 1] = xt_b2[6 * g + 0]
            xt_b2[6 * g + 2] = dsel
            xt_b2[6 * g + 3] = dsel
            xt_b2[6 * g + 4] = sel
            xt_b2[6 * g + 5] = sel

        im = {"rtall": rtall}
        xts = np.zeros((128, 384), np.float16)
        xts[:, 0:128] = np.eye(128, dtype=np.float16)
        xts[0:KA, 128:256] = xt_a
        xts[KA:2 * KA, 128:256] = xt_a
        xts[2 * KA:2 * KA + KB, 256:384] = xt_b2
        im["xts"] = xts
        # colb: [128, 3*Wp] channel-major full expansion
        im["colb"] = np.repeat(colbc, BH, axis=1).transpose(1, 0, 2).reshape(
            128, 3 * Wp_pad)
        in_maps.append(im)

    # adaptive epsilon: the expanded b^2 quadratic can round slightly
    # negative (sqrt would NaN).  Emulate the matmul, find the most
    # negative excursion, and fold a safely-larger offset into the c0
    # rows.  Costs sqrt(eps) extra darkness only where dist ~ 0.
    b2qmin = 0.0
    for im in in_maps:
        xt = im["xts"][2 * KA:2 * KA + KB, 256:384].astype(np.float32)
        rb = im["rtall"][2 * KA:].astype(np.float32)
        b2qmin = min(b2qmin, float((xt.T @ rb).min()))
    eps = max(2e-5, -1.5 * b2qmin)
    epsh = np.float16(eps)
    epsl = np.float16(eps - np.float64(epsh))
    for im in in_maps:
        rt = im["rtall"]
        # add eps (split exactly) onto every live c0 pair; dead slots keep
        # zero color so a nonzero dd there is harmless, but restrict to
        # live ones anyway via the existing h-part nonzero mask union
        for g in range(NG):
            r = 2 * KA + 6 * g
            h64 = rt[r + 4].astype(np.float64) + float(epsh)
            l64 = rt[r + 5].astype(np.float64) + float(epsl)
            rt[r + 4] = h64.astype(np.float16)
            rt[r + 5] = (h64 - rt[r + 4].astype(np.float64)
                         + l64).astype(np.float16)

    meta = {
        "Wp_pad": Wp_pad, "ncols": ncols, "mmax": mmax,
        "lens": lens, "offs": offs.tolist(),
        "order": order, "m_core": m_core, "assign": assign,
    }
    return in_maps, meta


# ---------------------------------------------------------------- bass side

def _build_program(Wp_pad, ncols, mmax, lens, offs):
    import concourse.bacc as bacc
    import concourse.mybir as mybir
    from concourse import tile

    f16 = mybir.dt.float16
    f32 = mybir.dt.float32
    AF = mybir.ActivationFunctionType
    OP = mybir.AluOpType
    KA = 4 * NG
    KB = 6 * NG
    NSC = Wp_pad // SUPER

    KR = 2 * KA + KB
    nc = bacc.Bacc("TRN2", target_bir_lowering=False, debug=False,
                   num_devices=N_CORES)
    xts_d = nc.dram_tensor("xts", [128, 384], f16, kind="ExternalInput").ap()
    rtall_d = nc.dram_tensor("rtall", [KR, Wp_pad], f16,
                             kind="ExternalInput").ap()
    colb_d = nc.dram_tensor("colb", [128, 3 * Wp_pad], f16,
                            kind="ExternalInput").ap()
    out_d = nc.dram_tensor("out", [128, 3 * ncols], f16,
                           kind="ExternalOutput").ap()

    with tile.TileContext(nc) as tc:
        with (
            tc.tile_pool(name="const", bufs=1) as constp,
            tc.tile_pool(name="work", bufs=8) as workp,
            tc.tile_pool(name="psA", bufs=2, space="PSUM") as psumA,
            tc.tile_pool(name="psB", bufs=4, space="PSUM") as psumB,
        ):
            xts = constp.tile([128, 384], f16)
            ident = xts[:, 0:128]
            xt_al = xts[0:KA, 128:256]
            xt_an = xts[KA:2 * KA, 128:256]
            xt_b2 = xts[2 * KA:2 * KA + KB, 256:384]

            rtall = constp.tile([KR, Wp_pad], f16)
            colb = constp.tile([128, 3 * Wp_pad], f16)
            vint = constp.tile([128, 3 * Wp_pad], f16)

            # preload the combined relu/sqrt ACT table before data arrives
            dmy = workp.tile([1, 16], f16, tag="dmy")
            nc.gpsimd.memset(dmy[:], 0.0)
            nc.scalar.activation(dmy[:], dmy[:], AF.Sqrt)

            # warm the PE clock gate on a junk tile during the DMA dead
            # zone so real matmuls start at full clock
            junk = constp.tile([128, 512], f16)
            nc.gpsimd.memset(junk[:], 0.0)
            pwarm = psumA.tile([128, 1024], f32, tag="pp")
            for _ in range(7):
                nc.tensor.matmul(pwarm[:, 0:512], junk[:, 0:128], junk[:])

            # single SP issue stream in consumption order: chunk-0 rt
            # slice, lhsT, chunk-0 colors, then the rest per chunk
            colb3 = colb[:].rearrange("p (c w) -> p c w", c=3)
            colb3_d = colb_d[:].rearrange("p (c w) -> p c w", c=3)
            nc.sync.dma_start(rtall[:, 0:SUPER], rtall_d[:, 0:SUPER])
            nc.sync.dma_start(xts[:], xts_d[:])
            nc.sync.dma_start(colb3[:, :, 0:SUPER], colb3_d[:, :, 0:SUPER])
            for sc in range(1, NSC):
                sl = slice(sc * SUPER, (sc + 1) * SUPER)
                nc.sync.dma_start(rtall[:, sl], rtall_d[:, sl])
                nc.sync.dma_start(colb3[:, :, sl], colb3_d[:, :, sl])

            # fold l becomes runnable once the chunk covering its src
            # range completes; emit right after that chunk's mults unless
            # it is the last chunk (keep the final critical tail clean)
            fold_after = {}
            for l in range(1, mmax):
                hi = offs[l] + lens[l]
                rdy = (hi - 1) // SUPER
                if rdy <= NSC - 2:
                    fold_after.setdefault(rdy, []).append(l)

            for sc in range(NSC):
                sl = slice(sc * SUPER, (sc + 1) * SUPER)
                palpa = psumA.tile([128, 1024], f32, tag="pp")
                pb2 = psumB.tile([128, SUPER], f32, tag="pb")
                nc.tensor.matmul(palpa[:, 0:512], xt_al, rtall[0:KA, sl])
                nc.tensor.matmul(palpa[:, 512:1024], xt_an,
                                 rtall[KA:2 * KA, sl])
                nc.tensor.matmul(pb2[:], xt_b2, rtall[2 * KA:KR, sl],
                                 start=True, stop=False)

                q12 = workp.tile([128, 1024], f16, tag="q12")
                mp = workp.tile([128, SUPER], f16, tag="mp")
                mp2 = workp.tile([128, SUPER], f16, tag="mp2")
                dd = workp.tile([128, SUPER], f16, tag="dd")

                nc.scalar.activation(q12[:], palpa[:], AF.Relu)
                nc.vector.tensor_tensor(mp[:], q12[:, 0:512],
                                        q12[:, 512:1024], op=OP.max)
                nc.vector.tensor_tensor(mp2[:], mp[:], mp[:], op=OP.mult)
                nc.tensor.matmul(pb2[:], ident, mp2[:],
                                 start=False, stop=True)
                nc.scalar.activation(dd[:], pb2[:], AF.Sqrt)
                ddm1 = workp.tile([128, SUPER], f16, tag="ddm1")
                nc.vector.tensor_scalar(ddm1[:], dd[:], 1.0, None,
                                        op0=OP.subtract)
                for ch in range(3):
                    nc.vector.tensor_tensor(
                        vint[:, ch * Wp_pad + sc * SUPER:
                             ch * Wp_pad + (sc + 1) * SUPER],
                        ddm1[:],
                        colb[:, ch * Wp_pad + sc * SUPER:
                             ch * Wp_pad + (sc + 1) * SUPER],
                        op=OP.mult)
                # ready folds slot in behind this chunk's mults
                for l in fold_after.get(sc, []):
                    w = lens[l]
                    o = offs[l]
                    for ch in range(3):
                        base = ch * Wp_pad
                        nc.vector.tensor_tensor(
                            vint[:, base:base + w], vint[:, base:base + w],
                            vint[:, base + o:base + o + w], op=OP.min)

            # remaining folds (ranges touching the last chunk); emit each
            # channel's output DMA as soon as its folds are done
            done_l = {l for ls in fold_after.values() for l in ls}
            rest = [l for l in range(1, mmax) if l not in done_l]
            for ch in range(3):
                base = ch * Wp_pad
                for l in rest:
                    w = lens[l]
                    o = offs[l]
                    nc.vector.tensor_tensor(
                        vint[:, base:base + w], vint[:, base:base + w],
                        vint[:, base + o:base + o + w], op=OP.min)
                eng = (nc.sync, nc.scalar, nc.sync)[ch]
                eng.dma_start(
                    out_d[:, ch * ncols:(ch + 1) * ncols],
                    vint[:, base:base + ncols])

    nc.compile()
    return nc


# ---------------------------------------------------------------- entry

def kernel(strokes, thicknesses, colors):
    _install_ntff_hook()
    from concourse.bass_utils import run_bass_kernel_spmd

    strokes = np.asarray(strokes)
    thicknesses = np.asarray(thicknesses)
    colors = np.asarray(colors)

    in_maps, meta = _build_layout(strokes, thicknesses, colors)
    key = (meta["Wp_pad"], meta["ncols"], meta["mmax"],
           tuple(meta["lens"]))
    if key not in _PROG_CACHE:
        _PROG_CACHE[key] = _build_program(
            meta["Wp_pad"], meta["ncols"], meta["mmax"],
            meta["lens"], meta["offs"])
    nc = _PROG_CACHE[key]

    res = run_bass_kernel_spmd(nc, in_maps, list(range(N_CORES)))

    ncols = meta["ncols"]
    out = np.zeros((3, G, G), np.float32)
    rows = np.arange(128)
    for cidx in range(N_CORES):
        mc = meta["m_core"][cidx]
        ordc = meta["order"][cidx][:ncols]
        real = mc[ordc] > 0
        cols_real = ordc[real]
        row_ids = (meta["assign"][cidx][rows // BH] * BH + rows % BH)
        for ch in range(3):
            plane = res.results[cidx]["out"][:, ch * ncols:(ch + 1) * ncols]
            vals = np.maximum(0.0, -plane[:, real].astype(np.float32))
            out[ch][np.ix_(row_ids, cols_real)] = vals
    return out


if __name__ == "__main__":
    rng = np.random.default_rng(0)
    s = rng.random((N, 2, 4), np.float32)
    th = rng.random((N, 1), np.float32)
    co = rng.random((N, 3), np.float32)
    g = kernel(s, th, co)
    print("out", g.shape, g.dtype, g.min(), g.max())


# revision 5
# speedup vs baseline: 1.0176x; 1.0148x over previous
"""Bezier stroke renderer on 8 Trainium2 NeuronCores — canvas-major
depth-packed Bass/Tile SPMD kernel (v2).

Reference: 32 strokes x 16 segments rasterized on a 1024x1024 canvas;
per pixel/segment darkness = clip((2t - dist)/(2t), 0, 1), max over
segments, grid = max(grid, darkness*color) over strokes.

Design:
  - Canvas rows split into 128 bands of 8 rows; each core owns 16
    consecutive bands (one 128-row slab), one band per 8-partition group.
  - Per band, each segment yields a column window [c0,c1] (pad=2t+1).
    For a canvas column c, m(c) = max over the core's bands of the
    number of windows covering c.  Columns are sorted by m desc and the
    packed axis stores, level-major, one slot per (level l, column with
    m>l).  Every group contributes its own window coefficients at each
    slot; empty slots are dead (zero coeffs, zero color).
  - Distance math in the segment tangent frame scaled by 1/(2t), with
    per-band row centering (delta = x - band_x0 in [0,8)) so 2-way fp16
    coefficient splits reach ~2^-22 relative accuracy:
      a  = a1*delta + a2(c)        (tangent coordinate)
      al = a - L/(2t)
      b2q = c2*delta^2 + c1*delta + c0     (= b^2, quadratic in delta)
    TensorE computes a, al, b2q; overshoot mp = max(relu(al), relu(-a))
    (ACT relus, DVE max), mp^2 (DVE), accumulated onto the b2q PSUM bank
    by an identity matmul; ACT takes sqrt; DVE scalar_tensor_tensor
    writes (dd-1)*col_ch into three fp16 channel planes (4x DVE mode).
  - Composite: level-major layout makes each level a prefix of the
    sorted columns, so min-compositing is mmax-1 in-place fp16 MIN ops
    per channel.  Output = level-0 block, fp16; host does relu(-x),
    scatter to canvas, fp32.
"""

import sys
import types
import contextlib
import ctypes

sys.path.insert(0, "/opt/trn_rl_repo")

import numpy as np

G = 1024
P = 16
N = 32
N_CORES = 8
BH = 16                # band height (rows)
NB = G // BH           # 64 bands
NG = 128 // BH         # 8 bands (groups) per core
SUPER = 512            # columns per PSUM chunk (1 bank per quantity)
COLBUF_BCAST = True    # expand colors on-chip via 0-stride DMA (else HBM full)

_PROG_CACHE = {}
_HOOK_INSTALLED = False


def _install_ntff_hook():
    global _HOOK_INSTALLED
    if _HOOK_INSTALLED:
        return
    _HOOK_INSTALLED = True
    try:
        import antenv
        mod = types.ModuleType("antenv.axon_hooks")
        holder = [None]
        mod.set_axon_ntff_profile_hook = lambda h: holder.__setitem__(0, h)
        mod.get_axon_ntff_profile_hook = lambda: holder[0]
        sys.modules["antenv.axon_hooks"] = mod
        antenv.axon_hooks = mod

        lib = ctypes.CDLL("/opt/axon/libaxon_pjrt.so")
        if not hasattr(lib, "axon_start_nrt_profile"):
            return
        lib.axon_start_nrt_profile.argtypes = [
            ctypes.POINTER(ctypes.c_int64),
            ctypes.c_size_t,
        ]
        lib.axon_start_nrt_profile.restype = ctypes.c_int64
        lib.axon_stop_nrt_profile.argtypes = [ctypes.c_char_p]
        lib.axon_stop_nrt_profile.restype = ctypes.c_int64

        @contextlib.contextmanager
        def _hook(output_dir, device_ids):
            import jax
            jax.devices()
            if device_ids:
                ids = (ctypes.c_int64 * len(device_ids))(*device_ids)
                rc = lib.axon_start_nrt_profile(ids, len(device_ids))
            else:
                rc = lib.axon_start_nrt_profile(None, 0)
            if rc != 0:
                raise RuntimeError(f"axon_start_nrt_profile rc={rc}")
            try:
                yield
            finally:
                n = lib.axon_stop_nrt_profile(str(output_dir).encode())
                print(f"profile: {n} file(s) written to {output_dir}",
                      file=sys.stderr)

        mod.set_axon_ntff_profile_hook(_hook)
    except Exception:
        pass


# ---------------------------------------------------------------- host side

def _bezier_weights_f32(p):
    t = np.arange(p, dtype=np.float64)
    w1 = (p - t) ** 3 / p ** 3
    w2 = 3 * (p - t) ** 2 * t / p ** 3
    w3 = 3 * (p - t) * t ** 2 / p ** 3
    w4 = t ** 3 / p ** 3
    return np.stack([w1, w2, w3, w4]).astype(np.float32)


def _polylines(strokes):
    W = _bezier_weights_f32(P)
    s = strokes.astype(np.float32)
    pts, derivs = s[:, :, :2], s[:, :, 2:]
    p1, p2 = pts[:, :-1], (pts + derivs)[:, :-1]
    p3, p4 = (pts - derivs)[:, 1:], pts[:, 1:]
    cp = np.stack([p1, p2, p3, p4], axis=3)
    sp = np.einsum("nsdk,kp->nspd", cp, W).astype(np.float32)
    sp = sp.reshape(s.shape[0], -1, 2)
    poly = np.concatenate([sp, pts[:, -1:, :]], axis=1).astype(np.float32)
    return (poly * np.float32(G)).astype(np.float64)


def _build_layout(strokes, thicknesses, colors):
    """Window extraction, per-core depth profiles, harmonized level
    layout, and all device tables."""
    poly = _polylines(strokes)
    t = np.maximum(thicknesses.astype(np.float32) * np.float32(2.0)
                   + np.float32(0.5), np.float32(0.5))[:, 0]
    col = np.clip(colors.astype(np.float32), 0.0, 1.0)
    t64 = t.astype(np.float64)
    pad = 2.0 * t64 + 1.0

    # windows per band: list of (n, iseg, c0, c1)
    wins = [[] for _ in range(NB)]
    for n in range(N):
        pn = poly[n]
        for i in range(P):
            v, w = pn[i], pn[i + 1]
            xlo, xhi = min(v[0], w[0]) - pad[n], max(v[0], w[0]) + pad[n]
            b0 = max(0, int(np.floor(xlo / BH)))
            b1 = min(NB - 1, int(np.floor(xhi / BH)))
            dx = w[0] - v[0]
            for b in range(b0, b1 + 1):
                x0, x1 = BH * b, BH * b + BH - 1
                lo_x, hi_x = x0 - pad[n], x1 + pad[n]
                if abs(dx) < 1e-12:
                    if v[0] < lo_x or v[0] > hi_x:
                        continue
                    s0, s1 = 0.0, 1.0
                else:
                    sa, sb = (lo_x - v[0]) / dx, (hi_x - v[0]) / dx
                    s0 = max(0.0, min(sa, sb))
                    s1 = min(1.0, max(sa, sb))
                    if s0 > s1:
                        continue
                ya = v[1] + s0 * (w[1] - v[1])
                yb = v[1] + s1 * (w[1] - v[1])
                c0 = max(0.0, min(ya, yb) - pad[n])
                c1 = min(G - 1.0, max(ya, yb) + pad[n])
                if c1 < c0:
                    continue
                wins[b].append((n, i, int(np.floor(c0)), int(np.ceil(c1))))

    # per-band, per-column window lists
    colwins = [[[] for _ in range(G)] for _ in range(NB)]
    for b in range(NB):
        for (n, i, c0, c1) in wins[b]:
            for c in range(c0, c1 + 1):
                colwins[b][c].append((n, i))

    # band depth profiles; assign bands to cores to minimize the
    # harmonized packed width (hill-climb from consecutive slabs)
    band_d = np.zeros((NB, G), np.int32)
    for b in range(NB):
        for c in range(G):
            band_d[b, c] = len(colwins[b][c])

    assign = np.arange(NB).reshape(N_CORES, NG)

    def harmonized_wp(asg):
        prof = np.sort(band_d[asg].max(axis=1), axis=1)[:, ::-1]
        return int(prof.max(axis=0).sum())

    rng = np.random.default_rng(12345)
    best = harmonized_wp(assign)
    best_assign = assign.copy()
    cur = best
    temp = 60.0
    for it in range(40000):
        temp *= 0.99985
        c1i, c2i = rng.integers(0, N_CORES, 2)
        if c1i == c2i:
            continue
        g1, g2 = rng.integers(0, NG, 2)
        assign[c1i, g1], assign[c2i, g2] = assign[c2i, g2], assign[c1i, g1]
        cand = harmonized_wp(assign)
        if cand <= cur or rng.random() < np.exp(-(cand - cur) / max(temp, 1e-9)):
            cur = cand
            if cand < best:
                best = cand
                best_assign = assign.copy()
        else:
            assign[c1i, g1], assign[c2i, g2] = (assign[c2i, g2],
                                                assign[c1i, g1])
    assign = best_assign

    m_core = np.zeros((N_CORES, G), np.int32)
    for cidx in range(N_CORES):
        m_core[cidx] = band_d[assign[cidx]].max(axis=0)

    # per-core column order: sort by m desc (stable)
    order = [np.argsort(-m_core[c], kind="stable") for c in range(N_CORES)]
    sorted_m = np.stack([m_core[c][order[c]] for c in range(N_CORES)])
    common = sorted_m.max(axis=0)           # harmonized profile
    ncols = int((common > 0).sum())
    mmax = int(common.max())
    # level lengths: len_l = #cols with common > l
    lens = [int((common > l).sum()) for l in range(mmax)]
    offs = np.concatenate([[0], np.cumsum(lens)]).astype(np.int64)
    Wp = int(offs[-1])
    Wp_pad = ((Wp + SUPER - 1) // SUPER) * SUPER

    # device tables per core
    KA = 4 * NG            # rows for a and al quantities
    KB = 6 * NG            # rows for b^2 quantity
    in_maps = []
    for cidx in range(N_CORES):
        ordc = order[cidx]
        # entry collection: (g, level, sortidx, n, iseg, canvascol)
        gs, ls, sis, ns, isegs, cs = [], [], [], [], [], []
        for g in range(NG):
            b = int(assign[cidx][g])
            for si in range(ncols):
                c = int(ordc[si])
                lst = colwins[b][c]
                for l, (n, i) in enumerate(lst):
                    gs.append(g); ls.append(l); sis.append(si)
                    ns.append(n); isegs.append(i); cs.append(c)
        gs = np.array(gs); ls = np.array(ls); sis = np.array(sis)
        ns = np.array(ns); isegs = np.array(isegs); cs = np.array(cs, np.float64)
        pos = offs[ls] + sis               # packed position per entry

        v = poly[ns, isegs]                # (E,2)
        w = poly[ns, isegs + 1]
        dxy = w - v
        L = np.hypot(dxy[:, 0], dxy[:, 1])
        safe = L > 1e-9
        taux = np.where(safe, dxy[:, 0] / np.where(safe, L, 1.0), 1.0)
        tauy = np.where(safe, dxy[:, 1] / np.where(safe, L, 1.0), 0.0)
        Leff = np.where(safe, L, 0.0)
        nux, nuy = -tauy, taux
        i2t = 1.0 / (2.0 * t64[ns])
        x0 = (assign[cidx][gs] * BH).astype(np.float64)  # band start row

        av = v[:, 0] * taux + v[:, 1] * tauy
        bv = v[:, 0] * nux + v[:, 1] * nuy
        a1 = taux * i2t
        a2 = (cs * tauy - av) * i2t + a1 * x0            # centered const
        al2 = a2 - Leff * i2t
        b1 = nux * i2t
        b20 = (cs * nuy - bv) * i2t + b1 * x0
        c2 = b1 * b1
        c1 = 2.0 * b1 * b20
        c0_ = b20 * b20

        def split2(vv):
            h = vv.astype(np.float16)
            lo = (vv - h.astype(np.float64)).astype(np.float16)
            return h, lo

        # rtall rows: [0:KA] a-L coeffs, [KA:2KA] NEGATED a coeffs (so a
        # single +relu serves both), [2KA:2KA+KB] b^2 quadratic coeffs
        rtall = np.zeros((2 * KA + KB, Wp_pad), np.float16)
        colbc = np.zeros((3, NG, Wp_pad), np.float16)

        a1h, a1l = split2(a1)
        na1h, na1l = split2(-a1)
        na2h, na2l = split2(-a2)
        al2h, al2l = split2(al2)
        c2h, c2l = split2(c2)
        c1h, c1l = split2(c1)
        c0h, c0l = split2(c0_)
        rtall[4 * gs + 0, pos] = a1h
        rtall[4 * gs + 1, pos] = a1l
        rtall[4 * gs + 2, pos] = al2h
        rtall[4 * gs + 3, pos] = al2l
        rtall[KA + 4 * gs + 0, pos] = na1h
        rtall[KA + 4 * gs + 1, pos] = na1l
        rtall[KA + 4 * gs + 2, pos] = na2h
        rtall[KA + 4 * gs + 3, pos] = na2l
        rtall[2 * KA + 6 * gs + 0, pos] = c2h
        rtall[2 * KA + 6 * gs + 1, pos] = c2l
        rtall[2 * KA + 6 * gs + 2, pos] = c1h
        rtall[2 * KA + 6 * gs + 3, pos] = c1l
        rtall[2 * KA + 6 * gs + 4, pos] = c0h
        rtall[2 * KA + 6 * gs + 5, pos] = c0l
        colv = col[ns]                      # (E,3) f32
        for ch in range(3):
            colbc[ch, gs, pos] = colv[:, ch].astype(np.float16)

        # lhsT tables
        dl = np.arange(128) % BH            # delta per partition
        xt_a = np.zeros((KA, 128), np.float16)
        xt_b2 = np.zeros((KB, 128), np.float16)
        for g in range(NG):
            sel = np.zeros(128, np.float16)
            sel[g * BH:(g + 1) * BH] = 1.0
            dsel = (dl * sel).astype(np.float16)
            xt_a[4 * g + 0] = dsel
            xt_a[4 * g + 1] = dsel
            xt_a[4 * g + 2] = sel
            xt_a[4 * g + 3] = sel
            xt_b2[6 * g + 0] = (dl * dl * sel).astype(np.float16)
            xt_b2[6 * g + 1] = xt_b2[6 * g + 0]
            xt_b2[6 * g + 2] = dsel
            xt_b2[6 * g + 3] = dsel
            xt_b2[6 * g + 4] = sel
            xt_b2[6 * g + 5] = sel

        im = {"rtall": rtall}
        xts = np.zeros((128, 384), np.float16)
        xts[:, 0:128] = np.eye(128, dtype=np.float16)
        xts[0:KA, 128:256] = xt_a
        xts[KA:2 * KA, 128:256] = xt_a
        xts[2 * KA:2 * KA + KB, 256:384] = xt_b2
        im["xts"] = xts
        # colb: [128, 3*Wp] channel-major full expansion
        im["colb"] = np.repeat(colbc, BH, axis=1).transpose(1, 0, 2).reshape(
            128, 3 * Wp_pad)
        in_maps.append(im)

    # adaptive epsilon: the expanded b^2 quadratic can round slightly
    # negative (sqrt would NaN).  Emulate the matmul, find the most
    # negative excursion, and fold a safely-larger offset into the c0
    # rows.  Costs sqrt(eps) extra darkness only where dist ~ 0.
    b2qmin = 0.0
    for im in in_maps:
        xt = im["xts"][2 * KA:2 * KA + KB, 256:384].astype(np.float32)
        rb = im["rtall"][2 * KA:].astype(np.float32)
        b2qmin = min(b2qmin, float((xt.T @ rb).min()))
    eps = max(2e-5, -1.5 * b2qmin)
    epsh = np.float16(eps)
    epsl = np.float16(eps - np.float64(epsh))
    for im in in_maps:
        rt = im["rtall"]
        # add eps (split exactly) onto every live c0 pair; dead slots keep
        # zero color so a nonzero dd there is harmless, but restrict to
        # live ones anyway via the existing h-part nonzero mask union
        for g in range(NG):
            r = 2 * KA + 6 * g
            h64 = rt[r + 4].astype(np.float64) + float(epsh)
            l64 = rt[r + 5].astype(np.float64) + float(epsl)
            rt[r + 4] = h64.astype(np.float16)
            rt[r + 5] = (h64 - rt[r + 4].astype(np.float64)
                         + l64).astype(np.float16)

    meta = {
        "Wp_pad": Wp_pad, "ncols": ncols, "mmax": mmax,
        "lens": lens, "offs": offs.tolist(),
        "order": order, "m_core": m_core, "assign": assign,
    }
    return in_maps, meta


# ---------------------------------------------------------------- bass side

def _build_program(Wp_pad, ncols, mmax, lens, offs):
    import concourse.bacc as bacc
    import concourse.mybir as mybir
    from concourse import tile

    f16 = mybir.dt.float16
    f32 = mybir.dt.float32
    AF = mybir.ActivationFunctionType
    OP = mybir.AluOpType
    KA = 4 * NG
    KB = 6 * NG
    NSC = Wp_pad // SUPER

    KR = 2 * KA + KB
    nc = bacc.Bacc("TRN2", target_bir_lowering=False, debug=False,
                   num_devices=N_CORES)
    xts_d = nc.dram_tensor("xts", [128, 384], f16, kind="ExternalInput").ap()
    rtall_d = nc.dram_tensor("rtall", [KR, Wp_pad], f16,
                             kind="ExternalInput").ap()
    colb_d = nc.dram_tensor("colb", [128, 3 * Wp_pad], f16,
                            kind="ExternalInput").ap()
    out_d = nc.dram_tensor("out", [128, 3 * ncols], f16,
                           kind="ExternalOutput").ap()

    with tile.TileContext(nc) as tc:
        with (
            tc.tile_pool(name="const", bufs=1) as constp,
            tc.tile_pool(name="work", bufs=8) as workp,
            tc.tile_pool(name="psA", bufs=2, space="PSUM") as psumA,
            tc.tile_pool(name="psB", bufs=4, space="PSUM") as psumB,
        ):
            xts = constp.tile([128, 384], f16)
            ident = xts[:, 0:128]
            xt_al = xts[0:KA, 128:256]
            xt_an = xts[KA:2 * KA, 128:256]
            xt_b2 = xts[2 * KA:2 * KA + KB, 256:384]

            rtall = constp.tile([KR, Wp_pad], f16)
            colb = constp.tile([128, 3 * Wp_pad], f16)
            vint = constp.tile([128, 3 * Wp_pad], f16)

            # preload the combined relu/sqrt ACT table before data arrives
            dmy = workp.tile([1, 16], f16, tag="dmy")
            nc.gpsimd.memset(dmy[:], 0.0)
            nc.scalar.activation(dmy[:], dmy[:], AF.Sqrt)

            # warm the PE clock gate on a junk tile during the DMA dead
            # zone so real matmuls start at full clock
            junk = constp.tile([128, 512], f16)
            nc.gpsimd.memset(junk[:], 0.0)
            pwarm = psumA.tile([128, 1024], f32, tag="pp")
            for _ in range(7):
                nc.tensor.matmul(pwarm[:, 0:512], junk[:, 0:128], junk[:])

            # single SP issue stream in consumption order: chunk-0 rt
            # slice, lhsT, chunk-0 colors, then the rest per chunk
            colb3 = colb[:].rearrange("p (c w) -> p c w", c=3)
            colb3_d = colb_d[:].rearrange("p (c w) -> p c w", c=3)
            nc.sync.dma_start(rtall[0:2 * KA, 0:SUPER],
                              rtall_d[0:2 * KA, 0:SUPER])
            nc.sync.dma_start(xts[:], xts_d[:])
            nc.sync.dma_start(rtall[2 * KA:KR, 0:SUPER],
                              rtall_d[2 * KA:KR, 0:SUPER])
            nc.sync.dma_start(colb3[:, :, 0:SUPER], colb3_d[:, :, 0:SUPER])
            for sc in range(1, NSC):
                sl = slice(sc * SUPER, (sc + 1) * SUPER)
                nc.sync.dma_start(rtall[:, sl], rtall_d[:, sl])
                nc.sync.dma_start(colb3[:, :, sl], colb3_d[:, :, sl])

            # fold l becomes runnable once the chunk covering its src
            # range completes; emit right after that chunk's mults unless
            # it is the last chunk (keep the final critical tail clean)
            fold_after = {}
            for l in range(1, mmax):
                hi = offs[l] + lens[l]
                rdy = (hi - 1) // SUPER
                if rdy <= NSC - 2:
                    fold_after.setdefault(rdy, []).append(l)

            for sc in range(NSC):
                sl = slice(sc * SUPER, (sc + 1) * SUPER)
                palpa = psumA.tile([128, 1024], f32, tag="pp")
                pb2 = psumB.tile([128, SUPER], f32, tag="pb")
                nc.tensor.matmul(palpa[:, 0:512], xt_al, rtall[0:KA, sl])
                nc.tensor.matmul(palpa[:, 512:1024], xt_an,
                                 rtall[KA:2 * KA, sl])
                nc.tensor.matmul(pb2[:], xt_b2, rtall[2 * KA:KR, sl],
                                 start=True, stop=False)

                q12 = workp.tile([128, 1024], f16, tag="q12")
                mp = workp.tile([128, SUPER], f16, tag="mp")
                mp2 = workp.tile([128, SUPER], f16, tag="mp2")
                dd = workp.tile([128, SUPER], f16, tag="dd")

                nc.scalar.activation(q12[:], palpa[:], AF.Relu)
                nc.vector.tensor_tensor(mp[:], q12[:, 0:512],
                                        q12[:, 512:1024], op=OP.max)
                nc.vector.tensor_tensor(mp2[:], mp[:], mp[:], op=OP.mult)
                nc.tensor.matmul(pb2[:], ident, mp2[:],
                                 start=False, stop=True)
                nc.scalar.activation(dd[:], pb2[:], AF.Sqrt)
                ddm1 = workp.tile([128, SUPER], f16, tag="ddm1")
                nc.vector.tensor_scalar(ddm1[:], dd[:], 1.0, None,
                                        op0=OP.subtract)
                for ch in range(3):
                    nc.vector.tensor_tensor(
                        vint[:, ch * Wp_pad + sc * SUPER:
                             ch * Wp_pad + (sc + 1) * SUPER],
                        ddm1[:],
                        colb[:, ch * Wp_pad + sc * SUPER:
                             ch * Wp_pad + (sc + 1) * SUPER],
                        op=OP.mult)
                # ready folds slot in behind this chunk's mults
                for l in fold_after.get(sc, []):
                    w = lens[l]
                    o = offs[l]
                    for ch in range(3):
                        base = ch * Wp_pad
                        nc.vector.tensor_tensor(
                            vint[:, base:base + w], vint[:, base:base + w],
                            vint[:, base + o:base + o + w], op=OP.min)

            # remaining folds (ranges touching the last chunk); emit each
            # channel's output DMA as soon as its folds are done
            done_l = {l for ls in fold_after.values() for l in ls}
            rest = [l for l in range(1, mmax) if l not in done_l]
            # after the folds wider than wcut, the output range
            # [wcut:ncols] is final — ship it while small folds finish
            small = [l for l in rest if lens[l] <= ncols // 3]
            big = [l for l in rest if l not in small]
            wcut = max((lens[l] for l in small), default=0)
            for ch in range(3):
                base = ch * Wp_pad
                eng = (nc.sync, nc.scalar, nc.sync)[ch]
                for l in big:
                    w = lens[l]
                    o = offs[l]
                    nc.vector.tensor_tensor(
                        vint[:, base:base + w], vint[:, base:base + w],
                        vint[:, base + o:base + o + w], op=OP.min)
                if small and wcut < ncols:
                    eng.dma_start(
                        out_d[:, ch * ncols + wcut:(ch + 1) * ncols],
                        vint[:, base + wcut:base + ncols])
                for l in small:
                    w = lens[l]
                    o = offs[l]
                    nc.vector.tensor_tensor(
                        vint[:, base:base + w], vint[:, base:base + w],
                        vint[:, base + o:base + o + w], op=OP.min)
                hi = min(wcut, ncols) if small else ncols
                eng.dma_start(out_d[:, ch * ncols:ch * ncols + hi],
                              vint[:, base:base + hi])

    nc.compile()
    return nc


# ---------------------------------------------------------------- entry

def kernel(strokes, thicknesses, colors):
    _install_ntff_hook()
    from concourse.bass_utils import run_bass_kernel_spmd

    strokes = np.asarray(strokes)
    thicknesses = np.asarray(thicknesses)
    colors = np.asarray(colors)

    in_maps, meta = _build_layout(strokes, thicknesses, colors)
    key = (meta["Wp_pad"], meta["ncols"], meta["mmax"],
           tuple(meta["lens"]))
    if key not in _PROG_CACHE:
        _PROG_CACHE[key] = _build_program(
            meta["Wp_pad"], meta["ncols"], meta["mmax"],
            meta["lens"], meta["offs"])
    nc = _PROG_CACHE[key]

    res = run_bass_kernel_spmd(nc, in_maps, list(range(N_CORES)))

    ncols = meta["ncols"]
    out = np.zeros((3, G, G), np.float32)
    rows = np.arange(128)
    for cidx in range(N_CORES):
        mc = meta["m_core"][cidx]
        ordc = meta["order"][cidx][:ncols]
        real = mc[ordc] > 0
        cols_real = ordc[real]
        row_ids = (meta["assign"][cidx][rows // BH] * BH + rows % BH)
        for ch in range(3):
            plane = res.results[cidx]["out"][:, ch * ncols:(ch + 1) * ncols]
            vals = np.maximum(0.0, -plane[:, real].astype(np.float32))
            out[ch][np.ix_(row_ids, cols_real)] = vals
    return out


if __name__ == "__main__":
    rng = np.random.default_rng(0)
    s = rng.random((N, 2, 4), np.float32)
    th = rng.random((N, 1), np.float32)
    co = rng.random((N, 3), np.float32)
    g = kernel(s, th, co)
    print("out", g.shape, g.dtype, g.min(), g.max())
